# revision 20
# baseline (speedup 1.0000x reference)
"""Trainium2 Bass kernel for double-softmax sparse attention.

reference math (per b,h):
    logits = (Q @ K^T) / sqrt(D) + mask * (-1e9)        # mask shared across h
    s1 = softmax(logits, axis=-1)
    s2 = softmax(s1 + dist + gnn, axis=-1)
    out = s2 @ V
returns (out, s2)

Sharding: data-parallel over batch B=16 -> 2 batches per core x 8 cores.
Each core handles 16 (b,h) units of S=512, D=64.
"""

import sys

if "/opt/trn_rl_repo" not in sys.path:
    sys.path.insert(0, "/opt/trn_rl_repo")

from contextlib import ExitStack

import numpy as np

import concourse.bacc as bacc
import concourse.bass as bass
import concourse.tile as tile
from concourse import mybir
from concourse.bass_utils import run_bass_kernel_spmd
from concourse.masks import make_identity

B, H, S, D = 16, 8, 512, 64
N_CORES = 8
BPC = B // N_CORES  # batches per core
NBH = BPC * H       # bh units per core
P = 128             # partitions
NT = S // P         # q-tiles per bh (4)

f32 = mybir.dt.float32
f32r = mybir.dt.float32r
bf16 = mybir.dt.bfloat16
ts = bass.ts
Alu = mybir.AluOpType
Act = mybir.ActivationFunctionType


def build(tc: tile.TileContext):
    nc = tc.nc
    q = nc.dram_tensor("q", [NBH, S, D], f32, kind="ExternalInput")
    k = nc.dram_tensor("k", [NBH, S, D], f32, kind="ExternalInput")
    v = nc.dram_tensor("v", [NBH, S, D], f32, kind="ExternalInput")
    dist = nc.dram_tensor("dist", [NBH, S, S], f32, kind="ExternalInput")
    gnn = nc.dram_tensor("gnn", [NBH, S, S], f32, kind="ExternalInput")
    mask = nc.dram_tensor("mask", [BPC, S, S], f32, kind="ExternalInput")
    out = nc.dram_tensor("out", [NBH, S, D], f32, kind="ExternalOutput")
    scores = nc.dram_tensor("scores", [NBH, S, S], f32, kind="ExternalOutput")

    # DRAM views tiled for 128-partition SBUF access
    q_t = q.ap().rearrange("n (t p) d -> n p t d", p=P)       # [NBH,128,4,64]
    k_t = k.ap().rearrange("n (t p) d -> n p t d", p=P)
    v_t = v.ap().rearrange("n (t p) d -> n p t d", p=P)
    dist_t = dist.ap().rearrange("n (t p) s -> n p t s", p=P)  # [NBH,128,4,512]
    gnn_t = gnn.ap().rearrange("n (t p) s -> n p t s", p=P)
    mask_t = mask.ap().rearrange("b (t p) s -> b p t s", p=P)  # [BPC,128,4,512]
    out_t = out.ap().rearrange("n (t p) d -> n p t d", p=P)
    scores_t = scores.ap().rearrange("n (t p) s -> n p t s", p=P)

    with ExitStack() as ctx:
        consts = ctx.enter_context(tc.tile_pool(name="consts", bufs=1))
        maskp = ctx.enter_context(tc.tile_pool(name="maskp", bufs=2))
        qkv = ctx.enter_context(tc.tile_pool(name="qkv", bufs=3))
        tposep = ctx.enter_context(tc.tile_pool(name="tposep", bufs=2, space="PSUM"))
        logitp = ctx.enter_context(tc.tile_pool(name="logitp", bufs=2, space="PSUM"))
        e2tp = ctx.enter_context(tc.tile_pool(name="e2tp", bufs=2, space="PSUM"))
        outp = ctx.enter_context(tc.tile_pool(name="outp", bufs=2, space="PSUM"))
        big = ctx.enter_context(tc.tile_pool(name="big", bufs=6))
        small = ctx.enter_context(tc.tile_pool(name="small", bufs=6))

        ident = consts.tile([P, P], f32)
        make_identity(nc, ident)
        ident_bf = consts.tile([P, P], bf16, tag="ident_bf")
        make_identity(nc, ident_bf)

        for b in range(BPC):
            # mask complement for this batch, shared across heads: 1 - mask
            maskc_f = maskp.tile([P, NT, S], f32)
            maskc = maskp.tile([P, NT, S], bf16, tag='maskc_bf')
            nc.sync.dma_start(out=maskc_f, in_=mask_t[b])
            nc.vector.tensor_scalar(
                maskc, maskc_f, -1.0, 1.0, op0=Alu.mult, op1=Alu.add
            )

            for h in range(H):
                bh = b * H + h
                # ---- load K, V, Q for this (b,h) ----
                k_sb = qkv.tile([P, NT, D], f32, tag="k")
                v_sb = qkv.tile([P, NT, D], f32, tag="v")
                q_sb = qkv.tile([P, NT, D], f32, tag="q")
                nc.sync.dma_start(out=k_sb, in_=k_t[bh])
                nc.sync.dma_start(out=v_sb, in_=v_t[bh])
                nc.sync.dma_start(out=q_sb, in_=q_t[bh])

                v_bf = qkv.tile([P, NT, D], bf16, tag="vbf")
                nc.vector.tensor_copy(v_bf, v_sb)

                # ---- transpose K and Q: [128,4,64] -> [64, 512] ----
                kT_ps = tposep.tile([D, NT, P], f32, tag="kqT")
                qT_ps = tposep.tile([D, NT, P], f32, tag="kqT")
                for c in range(NT):
                    nc.tensor.transpose(kT_ps[:, c], k_sb[:, c], ident)
                    nc.tensor.transpose(qT_ps[:, c], q_sb[:, c], ident)
                kT = qkv.tile([D, NT, P], bf16, tag="kTs")
                qT = qkv.tile([D, NT, P], bf16, tag="qTs")
                nc.any.tensor_copy(kT, kT_ps)
                nc.any.tensor_copy(qT, qT_ps)
                kT_r = kT.rearrange("d t p -> d (t p)")

                for i in range(NT):
                    # ---- prefetch dist/gnn (HWDGE), add on DVE ----
                    d_sb = big.tile([P, S], f32, tag="d")
                    g_sb = big.tile([P, S], f32, tag="g")
                    nc.sync.dma_start(out=d_sb, in_=dist_t[bh, :, i])
                    nc.sync.dma_start(out=g_sb, in_=gnn_t[bh, :, i])
                    dg = big.tile([P, S], bf16, tag="dg")
                    nc.vector.tensor_add(dg, d_sb, g_sb)

                    # ---- logits = Q_i @ K^T : [128, 512] in PSUM ----
                    lg = logitp.tile([P, S], f32, tag="lg")
                    nc.tensor.matmul(
                        lg, qT[:, i], kT_r, start=True, stop=True
                    )
                    # ---- e = exp(logits/8) straight from PSUM ----
                    e = big.tile([P, S], bf16, tag="e")
                    nc.scalar.activation(e, lg, Act.Exp, scale=0.125)
                    # ---- em = e * (1-mask), rowsum -> rs1 ----
                    em = big.tile([P, S], bf16, tag="em")
                    rs1 = small.tile([P, 1], f32, tag="rs1")
                    nc.vector.scalar_tensor_tensor(
                        out=em, in0=e, scalar=0.0, in1=maskc[:, i],
                        op0=Alu.bypass, op1=Alu.mult, accum_out=rs1,
                    )
                    r1 = small.tile([P, 1], f32, tag="r1")
                    nc.vector.reciprocal(r1, rs1)

                    # ---- s2 = em * r1 + dg ----
                    s2 = big.tile([P, S], bf16, tag="s2")
                    nc.vector.scalar_tensor_tensor(
                        out=s2, in0=em, scalar=r1, in1=dg,
                        op0=Alu.mult, op1=Alu.add,
                    )
                    # ---- e2 = exp(s2) (f32), rowsum -> rs2 ----
                    e2 = big.tile([P, S], bf16, tag="e2")
                    rs2 = small.tile([P, 1], f32, tag="rs2")
                    nc.scalar.activation(e2, s2, Act.Exp, accum_out=rs2)
                    r2 = small.tile([P, 1], f32, tag="r2")
                    nc.vector.reciprocal(r2, rs2)

                    # ---- scores output = e2 * r2 (f32) ----
                    sc = big.tile([P, S], f32, tag="sc")
                    nc.gpsimd.tensor_scalar_mul(sc, e2, r2)
                    nc.gpsimd.dma_start(out=scores_t[bh, :, i], in_=sc)

                    # ---- transpose e2 via PE (f32 -> PSUM), cast to bf16 ----
                    e2T_ps = e2tp.tile([P, NT, P], bf16, tag="e2Tp")
                    for c in range(NT):
                        nc.tensor.transpose(e2T_ps[:, c], e2[:, ts(c, P)], ident_bf)
                    e2T = big.tile([P, NT, P], bf16, tag="e2T")
                    nc.any.tensor_copy(e2T, e2T_ps)

                    # ---- out_i = (e2 @ V) * r2 ----
                    o_ps = outp.tile([P, D], f32, tag="o")
                    for c in range(NT):
                        nc.tensor.matmul(
                            o_ps, e2T[:, c], v_bf[:, c],
                            start=(c == 0), stop=(c == NT - 1),
                        )
                    o_sb = small.tile([P, D], f32, tag="osb")
                    nc.vector.tensor_scalar_mul(o_sb, o_ps, r2)
                    nc.gpsimd.dma_start(out=out_t[bh, :, i], in_=o_sb)


_COMPILED = None


def _get_compiled():
    global _COMPILED
    if _COMPILED is None:
        nc = bacc.Bacc(
            "TRN2", target_bir_lowering=False, debug=False, num_devices=N_CORES
        )
        with tile.TileContext(nc) as tc:
            build(tc)
        nc.compile()
        _COMPILED = nc
    return _COMPILED


def make_in_maps(query, key, value, gnn_adj, dist_score, mask):
    query = np.asarray(query, dtype=np.float32)
    key = np.asarray(key, dtype=np.float32)
    value = np.asarray(value, dtype=np.float32)
    gnn_adj = np.asarray(gnn_adj, dtype=np.float32)
    dist_score = np.asarray(dist_score, dtype=np.float32)
    mask = np.asarray(mask, dtype=np.float32)
    in_maps = []
    for c in range(N_CORES):
        bs = slice(c * BPC, (c + 1) * BPC)
        in_maps.append({
            "q": np.ascontiguousarray(query[bs].reshape(NBH, S, D)),
            "k": np.ascontiguousarray(key[bs].reshape(NBH, S, D)),
            "v": np.ascontiguousarray(value[bs].reshape(NBH, S, D)),
            "dist": np.ascontiguousarray(dist_score[bs].reshape(NBH, S, S)),
            "gnn": np.ascontiguousarray(gnn_adj[bs].reshape(NBH, S, S)),
            "mask": np.ascontiguousarray(mask[bs, 0]),
        })
    return in_maps


def gather_results(results):
    out = np.concatenate(
        [results[c]["out"].reshape(BPC, H, S, D) for c in range(N_CORES)], axis=0
    )
    scores = np.concatenate(
        [results[c]["scores"].reshape(BPC, H, S, S) for c in range(N_CORES)], axis=0
    )
    return out, scores


def kernel(query, key, value, gnn_adj, dist_score, mask):
    nc = _get_compiled()
    in_maps = make_in_maps(query, key, value, gnn_adj, dist_score, mask)
    res = run_bass_kernel_spmd(nc, in_maps, core_ids=list(range(N_CORES)))
    return gather_results(res.results)


# revision 22
# speedup vs baseline: 3.2762x; 3.2762x over previous
"""Trainium2 Bass kernel for double-softmax sparse attention.

reference math (per b,h):
    logits = (Q @ K^T) / sqrt(D) + mask * (-1e9)        # mask shared across h
    s1 = softmax(logits, axis=-1)
    s2 = softmax(s1 + dist + gnn, axis=-1)
    out = s2 @ V
returns (out, s2)

Sharding: data-parallel over batch B=16 -> 2 batches per core x 8 cores.
Each core handles 16 (b,h) units of S=512, D=64.
"""

import sys

if "/opt/trn_rl_repo" not in sys.path:
    sys.path.insert(0, "/opt/trn_rl_repo")

from contextlib import ExitStack

import numpy as np

import concourse.bacc as bacc
import concourse.bass as bass
import concourse.tile as tile
from concourse import mybir
from concourse.bass_utils import run_bass_kernel_spmd
from concourse.masks import make_identity

B, H, S, D = 16, 8, 512, 64
N_CORES = 8
BPC = B // N_CORES  # batches per core
NBH = BPC * H       # bh units per core
P = 128             # partitions
NT = S // P         # q-tiles per bh (4)

f32 = mybir.dt.float32
f32r = mybir.dt.float32r
bf16 = mybir.dt.bfloat16
ts = bass.ts
Alu = mybir.AluOpType
Act = mybir.ActivationFunctionType


def build(tc: tile.TileContext):
    nc = tc.nc
    q = nc.dram_tensor("q", [NBH, S, D], f32, kind="ExternalInput")
    k = nc.dram_tensor("k", [NBH, S, D], f32, kind="ExternalInput")
    v = nc.dram_tensor("v", [NBH, S, D], f32, kind="ExternalInput")
    dist = nc.dram_tensor("dist", [NBH, S, S], f32, kind="ExternalInput")
    gnn = nc.dram_tensor("gnn", [NBH, S, S], f32, kind="ExternalInput")
    mask = nc.dram_tensor("mask", [BPC, S, S], f32, kind="ExternalInput")
    out = nc.dram_tensor("out", [NBH, S, D], f32, kind="ExternalOutput")
    scores = nc.dram_tensor("scores", [NBH, S, S], f32, kind="ExternalOutput")

    # DRAM views tiled for 128-partition SBUF access
    q_t = q.ap().rearrange("n (t p) d -> n p t d", p=P)       # [NBH,128,4,64]
    k_t = k.ap().rearrange("n (t p) d -> n p t d", p=P)
    v_t = v.ap().rearrange("n (t p) d -> n p t d", p=P)
    dist_t = dist.ap().rearrange("n (t p) s -> n p t s", p=P)  # [NBH,128,4,512]
    gnn_t = gnn.ap().rearrange("n (t p) s -> n p t s", p=P)
    mask_t = mask.ap().rearrange("b (t p) s -> b p t s", p=P)  # [BPC,128,4,512]
    out_t = out.ap().rearrange("n (t p) d -> n p t d", p=P)
    scores_t = scores.ap().rearrange("n (t p) s -> n p t s", p=P)

    with ExitStack() as ctx:
        consts = ctx.enter_context(tc.tile_pool(name="consts", bufs=1))
        maskp = ctx.enter_context(tc.tile_pool(name="maskp", bufs=2))
        qkv = ctx.enter_context(tc.tile_pool(name="qkv", bufs=3))
        tposep = ctx.enter_context(tc.tile_pool(name="tposep", bufs=2, space="PSUM"))
        logitp = ctx.enter_context(tc.tile_pool(name="logitp", bufs=2, space="PSUM"))
        e2tp = ctx.enter_context(tc.tile_pool(name="e2tp", bufs=2, space="PSUM"))
        outp = ctx.enter_context(tc.tile_pool(name="outp", bufs=2, space="PSUM"))
        big = ctx.enter_context(tc.tile_pool(name="big", bufs=6))
        small = ctx.enter_context(tc.tile_pool(name="small", bufs=6))

        ident = consts.tile([P, P], f32)
        make_identity(nc, ident)
        ident_bf = consts.tile([P, P], bf16, tag="ident_bf")
        make_identity(nc, ident_bf)

        for b in range(BPC):
            # mask bias for this batch, shared across heads: -8e9 * mask
            # (becomes -1e9 after exp1's 0.125 scale)
            maskb_f = maskp.tile([P, NT, S], f32)
            maskb = maskp.tile([P, NT, S], bf16, tag="maskb_bf")
            nc.gpsimd.dma_start(out=maskb_f, in_=mask_t[b])
            nc.vector.tensor_scalar_mul(maskb, maskb_f, -8.0e9)

            for h in range(H):
                bh = b * H + h
                # ---- load K, V, Q for this (b,h) ----
                k_sb = qkv.tile([P, NT, D], f32, tag="k")
                v_sb = qkv.tile([P, NT, D], f32, tag="v")
                q_sb = qkv.tile([P, NT, D], f32, tag="q")
                nc.gpsimd.dma_start(out=k_sb, in_=k_t[bh])
                nc.gpsimd.dma_start(out=v_sb, in_=v_t[bh])
                nc.gpsimd.dma_start(out=q_sb, in_=q_t[bh])

                v_bf = qkv.tile([P, NT, D], bf16, tag="vbf")
                nc.vector.tensor_copy(v_bf, v_sb)

                # ---- transpose K and Q: [128,4,64] -> [64, 512] ----
                kT_ps = tposep.tile([D, NT, P], f32, tag="kqT")
                qT_ps = tposep.tile([D, NT, P], f32, tag="kqT")
                for c in range(NT):
                    nc.tensor.transpose(kT_ps[:, c], k_sb[:, c], ident)
                    nc.tensor.transpose(qT_ps[:, c], q_sb[:, c], ident)
                kT = qkv.tile([D, NT, P], bf16, tag="kTs")
                qT = qkv.tile([D, NT, P], bf16, tag="qTs")
                nc.any.tensor_copy(kT, kT_ps)
                nc.any.tensor_copy(qT, qT_ps)
                kT_r = kT.rearrange("d t p -> d (t p)")

                for i in range(NT):
                    # ---- prefetch dist/gnn (HWDGE), add on DVE ----
                    d_sb = big.tile([P, S], f32, tag="d")
                    g_sb = big.tile([P, S], f32, tag="g")
                    nc.gpsimd.dma_start(out=d_sb, in_=dist_t[bh, :, i])
                    nc.gpsimd.dma_start(out=g_sb, in_=gnn_t[bh, :, i])
                    dg = big.tile([P, S], bf16, tag="dg")
                    nc.vector.tensor_add(dg, d_sb, g_sb)

                    # ---- logits = maskb + Q_i @ K^T : [128, 512] in PSUM ----
                    lg = logitp.tile([P, S], f32, tag="lg")
                    nc.tensor.matmul(
                        lg, ident_bf, maskb[:, i], start=True, stop=False
                    )
                    nc.tensor.matmul(
                        lg, qT[:, i], kT_r, start=False, stop=True
                    )
                    # ---- e = exp(logits/8) from PSUM, rowsum -> rs1 ----
                    e = big.tile([P, S], bf16, tag="e")
                    rs1 = small.tile([P, 1], f32, tag="rs1")
                    nc.scalar.activation(e, lg, Act.Exp, scale=0.125,
                                         accum_out=rs1)
                    r1 = small.tile([P, 1], f32, tag="r1")
                    nc.vector.reciprocal(r1, rs1)

                    # ---- s2 = e * r1 + dg ----
                    s2 = big.tile([P, S], bf16, tag="s2")
                    nc.vector.scalar_tensor_tensor(
                        out=s2, in0=e, scalar=r1, in1=dg,
                        op0=Alu.mult, op1=Alu.add,
                    )
                    # ---- e2 = exp(s2) (f32), rowsum -> rs2 ----
                    e2 = big.tile([P, S], bf16, tag="e2")
                    rs2 = small.tile([P, 1], f32, tag="rs2")
                    nc.scalar.activation(e2, s2, Act.Exp, accum_out=rs2)
                    r2 = small.tile([P, 1], f32, tag="r2")
                    nc.vector.reciprocal(r2, rs2)

                    # ---- scores output = e2 * r2 (f32) ----
                    sc = big.tile([P, S], f32, tag="sc")
                    nc.vector.tensor_scalar_mul(sc, e2, r2)
                    nc.sync.dma_start(out=scores_t[bh, :, i], in_=sc)

                    # ---- transpose e2 via PE (f32 -> PSUM), cast to bf16 ----
                    e2T_ps = e2tp.tile([P, NT, P], bf16, tag="e2Tp")
                    for c in range(NT):
                        nc.tensor.transpose(e2T_ps[:, c], e2[:, ts(c, P)], ident_bf)
                    e2T = big.tile([P, NT, P], bf16, tag="e2T")
                    nc.any.tensor_copy(e2T, e2T_ps)

                    # ---- out_i = (e2 @ V) * r2 ----
                    o_ps = outp.tile([P, D], f32, tag="o")
                    for c in range(NT):
                        nc.tensor.matmul(
                            o_ps, e2T[:, c], v_bf[:, c],
                            start=(c == 0), stop=(c == NT - 1),
                        )
                    o_sb = small.tile([P, D], f32, tag="osb")
                    nc.vector.tensor_scalar_mul(o_sb, o_ps, r2)
                    nc.sync.dma_start(out=out_t[bh, :, i], in_=o_sb)


_COMPILED = None


def _get_compiled():
    global _COMPILED
    if _COMPILED is None:
        nc = bacc.Bacc(
            "TRN2", target_bir_lowering=False, debug=False, num_devices=N_CORES
        )
        with tile.TileContext(nc) as tc:
            build(tc)
        nc.compile()
        _COMPILED = nc
    return _COMPILED


def make_in_maps(query, key, value, gnn_adj, dist_score, mask):
    query = np.asarray(query, dtype=np.float32)
    key = np.asarray(key, dtype=np.float32)
    value = np.asarray(value, dtype=np.float32)
    gnn_adj = np.asarray(gnn_adj, dtype=np.float32)
    dist_score = np.asarray(dist_score, dtype=np.float32)
    mask = np.asarray(mask, dtype=np.float32)
    in_maps = []
    for c in range(N_CORES):
        bs = slice(c * BPC, (c + 1) * BPC)
        in_maps.append({
            "q": np.ascontiguousarray(query[bs].reshape(NBH, S, D)),
            "k": np.ascontiguousarray(key[bs].reshape(NBH, S, D)),
            "v": np.ascontiguousarray(value[bs].reshape(NBH, S, D)),
            "dist": np.ascontiguousarray(dist_score[bs].reshape(NBH, S, S)),
            "gnn": np.ascontiguousarray(gnn_adj[bs].reshape(NBH, S, S)),
            "mask": np.ascontiguousarray(mask[bs, 0]),
        })
    return in_maps


def gather_results(results):
    out = np.concatenate(
        [results[c]["out"].reshape(BPC, H, S, D) for c in range(N_CORES)], axis=0
    )
    scores = np.concatenate(
        [results[c]["scores"].reshape(BPC, H, S, S) for c in range(N_CORES)], axis=0
    )
    return out, scores


def kernel(query, key, value, gnn_adj, dist_score, mask):
    nc = _get_compiled()
    in_maps = make_in_maps(query, key, value, gnn_adj, dist_score, mask)
    res = run_bass_kernel_spmd(nc, in_maps, core_ids=list(range(N_CORES)))
    return gather_results(res.results)
